# revision 13
# baseline (speedup 1.0000x reference)
"""PixelPrototypeDistanceLoss on 8 Trainium2 NeuronCores.

Math: for each pixel p with label lb_p != 19:
    logit_p = emb_pixel_p . segment_queue[lb_p]
    loss = mean((1 - logit_p)^2)  over valid pixels

Trick: with onehot[c,p] = (lb_p == c) for c in [0,19), ignored pixels match
nothing, so
    sum_p valid*(1-logit)^2 = count - 2*S1 + S2
with count = sum(onehot), S1 = sum(sim*onehot), S2 = sum(sim^2*onehot),
all plain full reductions over the [C, N] similarity map -- no gather.

Sharding: batch dim across the 8 cores (one image each).  Per core:
  sim tiles [19, 512] computed as QT.T @ X with X = emb[b] reshaped [256, N].
  Four pixel-blocks stacked at partition offsets 0/32/64/96 (PE quadrant
  tile_position) so the DVE sees [128, C_g] blocks.  D=256 contraction via
  two fp8 matmuls accumulating in PSUM (DoubleRow would be 4x fewer PE
  cycles but its ISA encoding only allows dst partition 0, which defeats
  the quadrant stacking; quadrant concurrency is worth more).
  Onehot is built ON DEVICE (saves 0.5 MB/core of DMA = 11%): a tiny fp16
  matmul against rows (lb, lb^2, 1) with a quadratic SEL matrix emits
  lbq[32s+c, j] = 1 - (lb[j] - c)^2 exactly (all terms fp16-exact ints);
  ScalarE activation(Relu) turns that into the u8 onehot (GPSIMD cannot
  read PSUM, and this keeps the DVE to one op per group); DVE
  scalar_tensor_tensor fuses onehot*sim with the S1 row-sum; ScalarE
  activation(Square) accumulates S2.  Valid-count comes from one
  tensor_scalar(not_equal) over the raw labels.  The broadcast matmuls run
  while the x stream is still in flight (also keeps the PE HAM-warm).
Tail: one accumulator tile [128,13] -> single f32 ones-matmul partition
  reduce -> [1,13] copy -> one single-descriptor DMA.
Pipelining: emb is cast to fp8-e4m3 on the host (memory-bound problem),
  all input tiles are resident and their DMAs are issued upfront on one
  HWDGE queue, big transfers first, descending x tile sizes at the tail.
Host: sums the tiny per-core partial accumulators in f64.
"""

import numpy as np
import ml_dtypes

import concourse.bacc as bacc
import concourse.mybir as mybir
from concourse.tile import TileContext
from concourse import bass_utils

# Problem dims (hardcoded per harness contract).
B, D, H, W, C = 8, 256, 128, 128, 19
NPX = H * W          # 16384 pixels per core (one batch image)
NCORES = 8
IGNORE = 19.0

CP = 32              # padded class count (PE tile_position granularity)
F = 512              # max matmul out free dim (one PSUM bank of f32)
# x DMA tiles (pixel counts): big first for DMA efficiency, small at the
# end to shorten the post-stream serial drain
XTILES = [4096, 4096, 4096, 2048, 1024, 1024]
assert sum(XTILES) == NPX
NG = len(XTILES)
CGS = [n // 4 for n in XTILES]          # onehot/psum cols per group
OFFS = np.concatenate([[0], np.cumsum(CGS)]).tolist()  # col offsets
T0_COLS = NPX // 4                       # 4096

EMB_DT = mybir.dt.float8e4
EMB_NP = ml_dtypes.float8_e4m3
F16 = mybir.dt.float16
F16_NP = np.float16

META_COLS = 2 * CP + 4 + 128            # qt fp8 | ones f32 | labels u8
LBR_COLS = T0_COLS + 4 * CP             # stacked labels | SEL matrix
LBR_ROWS = 9                            # lb x4 stacks | lb^2 x4 | ones

_CACHE = {}


def _build():
    if "nc" in _CACHE:
        return _CACHE["nc"]
    nc = bacc.Bacc(
        "TRN2",
        target_bir_lowering=False,
        debug=False,
        enable_asserts=False,
    )
    # x packed on host as [128, 2*NPX]: group g's block at cols
    # [2*base_g, 2*base_g + 2*n); within a block col k*n+j = emb k-half
    x_t = nc.dram_tensor("x", [128, 2 * NPX], EMB_DT, kind="ExternalInput")
    # meta: cols 0:64 = qt fp8 bytes (col 32k+c = QT[128k+p, c]),
    # cols 64:68 = 1.0f, cols 68:196 = labels as u8
    meta_t = nc.dram_tensor("meta", [128, META_COLS], mybir.dt.uint8,
                            kind="ExternalInput")
    # lbr: rows 0-3 = group-stacked labels (row s col off_g+j =
    # lb[base_g + s*cg + j]), rows 4-7 = same stacked lb^2, row 8 = 1.0;
    # cols 4096: = quadratic SEL lhsT so that SEL.T @ lbr gives
    # 1 - (lb - c)^2: SEL[s, 32s+c] = 2c, SEL[4+s, 32s+c] = -1,
    # SEL[8, 32s+c] = 1 - c^2
    lbr_t = nc.dram_tensor("lbr", [LBR_ROWS, LBR_COLS], F16,
                           kind="ExternalInput")
    out_t = nc.dram_tensor("out", [1, 1 + 2 * NG], mybir.dt.float32,
                           kind="ExternalOutput")

    x = x_t.ap()
    meta = meta_t.ap()
    lbr = lbr_t.ap()
    out = out_t.ap()

    AO = mybir.AluOpType


    with TileContext(nc) as tc:
        with (
            tc.tile_pool(name="const", bufs=1) as cpool,
            tc.tile_pool(name="xp", bufs=1) as xpool,
            tc.tile_pool(name="scr", bufs=3) as spool,
            tc.tile_pool(name="acc", bufs=1) as apool,
            tc.tile_pool(name="psA", bufs=2, space="PSUM") as psa,
            tc.tile_pool(name="psB", bufs=2, space="PSUM") as psb,
        ):
            # all input tiles are resident; issue every DMA upfront on ONE
            # HWDGE queue (sync).  lbr + meta first (tiny, unblock PE).
            lbrt = cpool.tile([LBR_ROWS, LBR_COLS], F16)
            nc.sync.dma_start(lbrt[:, :], lbr[:, :])
            metat = cpool.tile([128, META_COLS], mybir.dt.uint8)
            nc.sync.dma_start(metat[:, :], meta[:, :])
            xt = {}
            base = 0
            for g, n in enumerate(XTILES):
                t = xpool.tile([128, 2 * n], EMB_DT, tag=f"xg{g}")
                nc.sync.dma_start(t[:, :], x[:, 2 * base:2 * base + 2 * n])
                xt[g] = t
                base += n

            qt_sb = metat[:, 0:2 * CP].bitcast(EMB_DT)
            ones_sb = metat[:, 2 * CP:2 * CP + 4].bitcast(mybir.dt.float32)
            lb_sb = metat[:, 2 * CP + 4:META_COLS]
            sel_sb = lbrt[:, T0_COLS:LBR_COLS]

            acc = apool.tile([128, 1 + 2 * NG], mybir.dt.float32)
            junk = apool.tile([128, 128], mybir.dt.float32)
            t0 = apool.tile([128, T0_COLS], mybir.dt.uint8)
            t2 = apool.tile([128, max(CGS)], mybir.dt.float32)

            # count of valid pixels (per partition; host sums).
            nc.vector.tensor_scalar(junk[:, :], lb_sb[:, :], IGNORE, None,
                                    AO.not_equal, AO.add,
                                    accum_out=acc[:, 0:1])

            def lbdiff_mm(g):
                # lbq[32s+c, j] = 1 - (lb[base+s*cg+j] - c)^2, via SEL.T@lbr
                cg = CGS[g]
                ps = psb.tile([128, cg], mybir.dt.float32, tag="psB")
                for m in range(0, cg, F):
                    fb = min(F, cg - m)
                    nc.tensor.matmul(
                        out=ps[:, m:m + fb],
                        lhsT=sel_sb,
                        rhs=lbrt[:, OFFS[g] + m:OFFS[g] + m + fb],
                        start=True, stop=True)
                return ps

            def cmp(g, ps):
                # onehot u8 <- Relu(1 - (lb-c)^2), frees the psB slot
                cg = CGS[g]
                nc.scalar.activation(t0[:, OFFS[g]:OFFS[g] + cg], ps[:, :],
                                     mybir.ActivationFunctionType.Relu)

            def sim_mm(g):
                cg = CGS[g]
                n = XTILES[g]
                ps = psa.tile([128, cg], mybir.dt.float32, tag="psA")
                for s in range(4):
                    for m in range(0, cg, F):
                        fb = min(F, cg - m)
                        for k in range(2):
                            col = k * n + s * cg + m
                            nc.tensor.matmul(
                                out=ps[CP * s:CP * (s + 1), m:m + fb],
                                lhsT=qt_sb[:, k * CP:(k + 1) * CP],
                                rhs=xt[g][:, col:col + fb],
                                start=(k == 0), stop=(k == 1),
                                tile_position=(0, CP * s))
                return ps

            def reduce_g(g, ps):
                cg = CGS[g]
                t1 = spool.tile([128, cg], mybir.dt.float32, tag="t1")
                # t1 = onehot * sim ; acc[:, 1+g] = row-sum(t1)
                nc.vector.scalar_tensor_tensor(
                    out=t1[:, :], in0=t0[:, OFFS[g]:OFFS[g] + cg],
                    scalar=1.0, in1=ps[:, :], op0=AO.mult, op1=AO.mult,
                    accum_out=acc[:, 1 + g:2 + g])
                # t2 = t1^2 = onehot*sim^2 ; acc[:, 1+NG+g] = row-sum
                nc.scalar.activation(
                    t2[:, 0:cg], t1[:, :],
                    mybir.ActivationFunctionType.Square,
                    accum_out=acc[:, 1 + NG + g:2 + NG + g])

            # interleave: lbdiff broadcasts run ahead (PE warm + t0 ready
            # before each x tile lands); sim groups consume x as it arrives
            pend = {}
            pend[0] = lbdiff_mm(0)
            pend[1] = lbdiff_mm(1)
            cmp(0, pend.pop(0))
            pend[2] = lbdiff_mm(2)
            cmp(1, pend.pop(1))
            for g in range(NG):
                if g + 3 <= NG - 1:
                    pend[g + 3] = lbdiff_mm(g + 3)
                if g + 2 in pend:
                    cmp(g + 2, pend.pop(g + 2))
                ps = sim_mm(g)
                reduce_g(g, ps)

            # partition-reduce the accumulators on the PE so the output is
            # one single-descriptor [1, 13] DMA
            ps_out = psb.tile([1, 1 + 2 * NG], mybir.dt.float32, tag="psB")
            nc.tensor.matmul(out=ps_out[0:1, :], lhsT=ones_sb[:, 0:1],
                             rhs=acc[:, :], start=True, stop=True)
            res = apool.tile([1, 1 + 2 * NG], mybir.dt.float32)
            nc.vector.tensor_copy(res[:, :], ps_out[0:1, :])
            nc.sync.dma_start(out[:, :], res[:, :])

    nc.compile()
    _CACHE["nc"] = nc
    return nc


def _prep_in_maps(emb, lb, segment_queue):
    emb = np.asarray(emb)
    lb = np.asarray(lb)
    q = np.asarray(segment_queue, dtype=np.float32)

    qt = np.zeros((D, CP), np.float32)
    qt[:, :C] = q.T
    # pack [2,128,CP] -> [128, 2*CP]: col 32k+c = QT[128k+p, c]
    qt = np.ascontiguousarray(
        qt.reshape(2, 128, CP).transpose(1, 0, 2).reshape(128, 2 * CP)
        .astype(EMB_NP))

    # quadratic SEL lhsT [9, 128]: col 32s+c computes
    # 2c*lb_s - lb_s^2 + (1 - c^2) = 1 - (lb_s - c)^2
    cs = np.tile(np.arange(CP, dtype=np.float32), 4)
    sel = np.zeros((LBR_ROWS, 128), np.float32)
    for s in range(4):
        sel[s, CP * s:CP * (s + 1)] = 2.0 * cs[CP * s:CP * (s + 1)]
        sel[4 + s, CP * s:CP * (s + 1)] = -1.0
    sel[8, :] = 1.0 - cs * cs

    in_maps = []
    for b in range(B):
        x8 = emb[b].reshape(2, 128, NPX).astype(EMB_NP)
        # pack per DMA tile: xb[p, 2*base + k*n + j] = x8[k, p, base + j]
        xb = np.empty((128, 2 * NPX), EMB_NP)
        base = 0
        for n in XTILES:
            blk = x8[:, :, base:base + n]            # [2, 128, n]
            xb[:, 2 * base:2 * base + 2 * n] = (
                blk.transpose(1, 0, 2).reshape(128, 2 * n))
            base += n
        lbf = lb[b].reshape(-1).astype(np.float32)

        meta = np.empty((128, META_COLS), np.uint8)
        meta[:, :2 * CP] = qt.view(np.uint8)
        meta[:, 2 * CP:2 * CP + 4] = (
            np.ones((128, 1), np.float32).view(np.uint8))
        meta[:, 2 * CP + 4:] = lbf.reshape(128, 128).astype(np.uint8)

        lbr = np.empty((LBR_ROWS, LBR_COLS), np.float32)
        base = 0
        for g, n in enumerate(XTILES):
            cg = CGS[g]
            stk = lbf[base:base + n].reshape(4, cg)
            lbr[0:4, OFFS[g]:OFFS[g] + cg] = stk
            lbr[4:8, OFFS[g]:OFFS[g] + cg] = stk * stk
            base += n
        lbr[8, 0:T0_COLS] = 1.0
        lbr[:, T0_COLS:] = sel

        in_maps.append({
            "x": xb,
            "meta": np.ascontiguousarray(meta),
            "lbr": np.ascontiguousarray(lbr.astype(F16_NP)),
        })
    return in_maps


def _reduce_outputs(results):
    cnt = 0.0
    s1 = 0.0
    s2 = 0.0
    for r in results:
        o = np.asarray(r["out"], dtype=np.float64)
        cnt += o[0, 0]
        s1 += o[0, 1:1 + NG].sum()
        s2 += o[0, 1 + NG:1 + 2 * NG].sum()
    num = cnt - 2.0 * s1 + s2
    return np.float32(num / cnt)


def run_on_cores(inputs, **kwargs):
    """Run the bass kernel on cores 0-7; returns (loss, BassKernelResults).

    The device occasionally reports a transient NRT_EXEC_UNIT_UNRECOVERABLE
    on a run that succeeds on immediate retry; retry a couple of times.
    """
    nc = _build()
    in_maps = _prep_in_maps(**inputs)
    last_err = None
    for _ in range(3):
        try:
            res = bass_utils.run_bass_kernel_spmd(
                nc, in_maps, core_ids=list(range(NCORES)), **kwargs)
            return _reduce_outputs(res.results), res
        except Exception as e:  # transient device wedge -> retry
            last_err = e
    raise last_err


def kernel(emb, lb, segment_queue):
    loss, _ = run_on_cores({"emb": emb, "lb": lb, "segment_queue": segment_queue})
    return loss
